# revision 25
# baseline (speedup 1.0000x reference)
"""DLSMN scatter-memory + cache self-attention kernel for Trainium2.

Data-parallel over batch: batch b runs on NeuronCore b (8 cores), no
collectives.  Inside one core (one batch):

  phase A: per 128-token tile of y (bf16): PE-transpose y -> yT chunks,
           fused matmuls [W_write | (W_slot,W_gate)] (bf16), gumbel-softmax
           routing via exp(logits*gamma - ln(-ln(u+eps)+eps)), weighted-
           scatter matmul (f32r) with leading ones columns so write-mass
           comes out of the same accumulation.
  phase B: slot update  upd = (1-g)*DECAY*old + g*updates/(mass+eps).
  phase C: PE-transpose cache2 (f32r data, bf16 identity) -> fp8 chunks
           laid out as DoubleRow d-pairs.
  phase D: q/k/v projections as fp8 DoubleRow matmuls (weights scaled x32
           to avoid fp8 subnormals; scales folded downstream).
  phase E: attention transposed: per (head-pair, 512-col chunk), s^T tiles
           via bf16 matmuls (512-wide moving), exp -> fp8 m-pair tiles;
           att@v and denominators as fp8 DoubleRow over m-pairs, consumed
           one pair late so the PE never stalls on the exp.
  phase F: pipelined one chunk behind E: o-projection fp8 DoubleRow +
           residual + layernorm (variance via ACT Square accumulate).
"""

import numpy as np

import concourse.bacc as bacc
import concourse.mybir as mybir
import concourse.tile as tile
from concourse.bass_utils import run_bass_kernel_spmd
from concourse.masks import make_identity

F32 = mybir.dt.float32
F32R = mybir.dt.float32r
BF16 = mybir.dt.bfloat16
F8 = mybir.dt.float8e4
AF = mybir.ActivationFunctionType
ALU = mybir.AluOpType
DR = mybir.MatmulPerfMode.DoubleRow

B = 8
S = 2048
D = 1024
DC = 512
K = 256
L = 8
H = 4
HD = 128
N = L * K
LAYER_IDX = 3
DECAY = 0.9
EPS = 1e-6
ST = S // 128  # 16 token tiles
NT = N // 128  # 16 slot tiles
DCH = D // 128  # 8 d_model chunks
CL = 512  # attention n-chunk length
NCH = N // CL  # 4 attention chunks
MM = NT // 2  # 8 m-tile pairs
ATT_SCALE = float(1.0 / np.sqrt(np.float32(HD)))
WSC = 32.0  # fp8 weight scale (avoids e4m3 subnormals)
AOSC = 64.0  # aoT fp8 scale
OINV = float(1.0 / (WSC * AOSC))  # o-proj descale (aoT' @ Wo')

_INPUT_SPECS = {
    "y": (S, D), "cache": (N, DC), "gumbel_u": (S, K),
    "W_gate": (D, 1), "b_gate": (1,), "W_slot": (D, K), "b_slot": (K,),
    "gamma": (1,), "W_write": (D, DC), "b_write": (DC,),
    "Wq": (DC, DC), "bq": (DC,), "Wk": (DC, DC), "bk": (DC,),
    "Wv": (DC, DC), "bv": (DC,), "Wo": (DC, DC), "bo": (DC,),
    "ln_g": (DC,), "ln_b": (DC,),
}


def _build():
    nc = bacc.Bacc("TRN2", target_bir_lowering=False, debug=False, num_devices=B)

    a = {
        name: nc.dram_tensor(name, list(shape), F32, kind="ExternalInput").ap()
        for name, shape in _INPUT_SPECS.items()
    }
    out_dram = nc.dram_tensor("out", [N, DC], F32, kind="ExternalOutput").ap()

    y3 = a["y"].rearrange("(t p) d -> p t d", p=128)
    gum3 = a["gumbel_u"].rearrange("(t p) k -> p t k", p=128)
    cache3 = a["cache"].rearrange("(t p) d -> p t d", p=128)
    out3 = out_dram.rearrange("(t p) d -> p t d", p=128)

    with tile.TileContext(nc) as tc:
        with (
            tc.tile_pool(name="const", bufs=1) as const,
            tc.tile_pool(name="cachep", bufs=1) as cachep,
        ):
            ident_bf = const.tile([128, 128], BF16)
            make_identity(nc, ident_bf)
            ident_f = const.tile([128, 128], F32)
            make_identity(nc, ident_f)
            ones_col2_f = const.tile([128, 2], F32)
            nc.vector.memset(ones_col2_f, 1.0)
            ones_row_bf = const.tile([1, DC], BF16)
            nc.vector.memset(ones_row_bf, 1.0)
            ones2_f8 = const.tile([128, 2, 32], F8)
            nc.vector.memset(ones2_f8, 1.0)
            eps8_t = const.tile([128, 1], F32)
            nc.vector.memset(eps8_t, 1e-8)
            eps5_t = const.tile([128, 1], F32)
            nc.vector.memset(eps5_t, 1e-5)
            gamma_t = const.tile([128, 1], F32)
            nc.sync.dma_start(out=gamma_t, in_=a["gamma"].unsqueeze(0).to_broadcast([128, 1]))
            lng_bc = const.tile([128, DC], F32)
            nc.sync.dma_start(out=lng_bc, in_=a["ln_g"].unsqueeze(0).to_broadcast([128, DC]))
            lnb_bc = const.tile([128, DC], F32)
            nc.sync.dma_start(out=lnb_bc, in_=a["ln_b"].unsqueeze(0).to_broadcast([128, DC]))
            bwr_bc = const.tile([128, DC], F32)
            nc.sync.dma_start(out=bwr_bc, in_=a["b_write"].unsqueeze(0).to_broadcast([128, DC]))
            bsg_row = const.tile([1, K + 2], BF16)
            nc.gpsimd.dma_start(out=bsg_row[:, 0:K], in_=a["b_slot"].unsqueeze(0))
            nc.gpsimd.dma_start(out=bsg_row[:, K:K + 1], in_=a["b_gate"].unsqueeze(0))
            nc.gpsimd.dma_start(out=bsg_row[:, K + 1:K + 2], in_=a["b_gate"].unsqueeze(0))
            # q/k per-head bias columns, pre-scaled by WSC (weights are x32)
            bq_raw = const.tile([128, H], F32)
            nc.gpsimd.dma_start(out=bq_raw, in_=a["bq"].rearrange("(h p) -> p h", p=128))
            bq_cols = const.tile([128, H], F32)
            nc.vector.tensor_scalar_mul(bq_cols, bq_raw, WSC)
            bk_raw = const.tile([128, H], F32)
            nc.gpsimd.dma_start(out=bk_raw, in_=a["bk"].rearrange("(h p) -> p h", p=128))
            bk_cols = const.tile([128, H], F32)
            nc.vector.tensor_scalar_mul(bk_cols, bk_raw, WSC)
            bv_cols = const.tile([128, H], BF16)
            nc.gpsimd.dma_start(out=bv_cols, in_=a["bv"].rearrange("(h p) -> p h", p=128))
            bo_row_f = const.tile([1, DC], F32)
            nc.gpsimd.dma_start(out=bo_row_f, in_=a["bo"].unsqueeze(0))

            cache_sb = cachep.tile([128, NT, DC], F32)

            # ---------------- phase A + B: selection & scatter write ------
            with (
                tc.tile_pool(name="wA", bufs=1) as wA,
                tc.tile_pool(name="pA", bufs=2) as pA,
                tc.tile_pool(name="pAs", bufs=3) as pAs,
                tc.tile_pool(name="psU", bufs=1, space="PSUM") as psU,
                tc.tile_pool(name="psA", bufs=1, space="PSUM") as psA,
                tc.tile_pool(name="psT", bufs=2, space="PSUM") as psT,
            ):
                wwr = wA.tile([128, DCH, DC], BF16)
                wsg = wA.tile([128, DCH, K + 2], BF16)

                # gumbel pre-pass: all Ln ops batched
                lnz_all = wA.tile([128, ST, K], F32)
                for i in range(ST):
                    gum = pA.tile([128, K], F32, tag="gum")
                    nc.sync.dma_start(out=gum, in_=gum3[:, i, :])
                    lnu = pAs.tile([128, K], F32, tag="lnu")
                    nc.scalar.activation(lnu, gum, AF.Ln, bias=eps8_t)
                    nc.scalar.activation(lnz_all[:, i, :], lnu, AF.Ln, bias=eps8_t,
                                         scale=-1.0)

                # persistent scatter accumulators: [ones|wv] x w -> [mass|updates]
                ps_ua = [psU.tile([128, K + 2], F32, name=f"ua{kc}", tag=f"ua{kc}")
                         for kc in range(2)]
                ps_ub = [psU.tile([128, K], F32, name=f"ub{kc}", tag=f"ub{kc}")
                         for kc in range(2)]

                pending = []

                def flush_updates():
                    while pending:
                        j, w_j, wv_j = pending.pop(0)
                        for kc in range(2):
                            lhs = w_j[:, kc * 128:(kc + 1) * 128]
                            nc.tensor.matmul(ps_ua[kc], lhs, wv_j[:, 0:K + 2],
                                             start=(j == 0), stop=(j == ST - 1))
                            nc.tensor.matmul(ps_ub[kc], lhs, wv_j[:, K + 2:DC + 2],
                                             start=(j == 0), stop=(j == ST - 1))

                for i in range(ST):
                    y_t = pA.tile([128, D], BF16, tag="y")
                    nc.gpsimd.dma_start(out=y_t, in_=y3[:, i, :])
                    if i == 0:
                        wwr3 = a["W_write"].rearrange("(c p) d -> p c d", p=128)
                        wsl3 = a["W_slot"].rearrange("(c p) k -> p c k", p=128)
                        for c in range(DCH):
                            nc.gpsimd.dma_start(out=wwr[:, c, :], in_=wwr3[:, c, :])
                            nc.gpsimd.dma_start(out=wsg[:, c, 0:K], in_=wsl3[:, c, :])
                        nc.gpsimd.dma_start(out=wsg[:, :, K:K + 1], in_=a["W_gate"].rearrange("(c p) o -> p c o", p=128))
                        nc.gpsimd.dma_start(out=wsg[:, :, K + 1:K + 2], in_=a["W_gate"].rearrange("(c p) o -> p c o", p=128))
                    if i == 1:
                        nc.sync.dma_start(out=cache_sb, in_=cache3)

                    # transpose y tile -> yT (8 chunks of [128d, 128s], bf16)
                    yT = pA.tile([128, D], BF16, tag="yT")
                    for g in range(2):
                        tr = psT.tile([128, 512], BF16, tag="tr")
                        for cc in range(4):
                            c = 4 * g + cc
                            nc.tensor.transpose(
                                tr[:, cc * 128:(cc + 1) * 128],
                                y_t[:, c * 128:(c + 1) * 128],
                                ident_bf,
                            )
                        nc.vector.tensor_copy(out=yT[:, g * 512:(g + 1) * 512], in_=tr)
                    flush_updates()

                    # fused write_vals / (logits, gate) matmuls (bf16)
                    ps_wv = psA.tile([128, DC], F32, tag="wv")
                    for c in range(DCH):
                        nc.tensor.matmul(
                            ps_wv, yT[:, c * 128:(c + 1) * 128], wwr[:, c, :],
                            start=(c == 0), stop=(c == DCH - 1),
                        )
                    ps_lg = psA.tile([128, K + 2], F32, tag="lg")
                    for c in range(DCH):
                        nc.tensor.matmul(
                            ps_lg, yT[:, c * 128:(c + 1) * 128], wsg[:, c, :],
                            start=(c == 0), stop=False,
                        )
                    nc.tensor.matmul(ps_lg, ones_row_bf[:, 0:128], bsg_row,
                                     start=False, stop=True)

                    # t = gamma*logits - lnz
                    t_sb = pAs.tile([128, K], F32, tag="tsb")
                    nc.vector.scalar_tensor_tensor(
                        out=t_sb, in0=ps_lg[:, 0:K], scalar=gamma_t, in1=lnz_all[:, i, :],
                        op0=ALU.mult, op1=ALU.subtract,
                    )

                    # scores = sigmoid(gate) = 1/(1+exp(-gate))
                    sc_e = pAs.tile([128, 1], F32, tag="sce")
                    nc.scalar.activation(sc_e, ps_lg[:, K:K + 1], AF.Exp, scale=-1.0)
                    sc1 = pAs.tile([128, 1], F32, tag="sc1")
                    nc.vector.tensor_scalar_add(sc1, sc_e, 1.0)
                    scores = pAs.tile([128, 1], F32, tag="scores")
                    nc.vector.reciprocal(scores, sc1)

                    # p_unnorm = exp(t), row-sum fused; w = p_unnorm*(scores/rowsum)
                    p_un = pAs.tile([128, K], F32, tag="pun")
                    rs = pAs.tile([128, 1], F32, tag="rs")
                    nc.scalar.activation(p_un, t_sb, AF.Exp, accum_out=rs)
                    rrs = pAs.tile([128, 1], F32, tag="rrs")
                    nc.vector.reciprocal(rrs, rs)
                    w_sb = pAs.tile([128, K], F32R, tag="wsb")
                    nc.vector.tensor_scalar(w_sb, p_un, scores, rrs,
                                            ALU.mult, ALU.mult)

                    # wv_sb = [ones | write_vals + b_write]
                    wv_sb = pAs.tile([128, DC + 2], F32R, tag="wvsb")
                    nc.vector.tensor_copy(out=wv_sb[:, 0:2], in_=ones_col2_f)
                    nc.vector.scalar_tensor_tensor(
                        out=wv_sb[:, 2:DC + 2], in0=ps_wv, scalar=1.0, in1=bwr_bc,
                        op0=ALU.mult, op1=ALU.add,
                    )
                    pending.append((i, w_sb, wv_sb))

                flush_updates()

                # ------- phase B: slot update, overwrite cache rows -------
                base_t = LAYER_IDX * K // 128  # n-tile 6
                for kc in range(2):
                    mass = pAs.tile([128, 1], F32, tag="mass")
                    nc.vector.tensor_copy(out=mass, in_=ps_ua[kc][:, 0:1])
                    m1 = pAs.tile([128, 1], F32, tag="m1")
                    nc.vector.tensor_scalar_add(m1, mass, EPS)
                    rm = pAs.tile([128, 1], F32, tag="rm")
                    nc.vector.reciprocal(rm, m1)
                    m2 = pAs.tile([128, 1], F32, tag="m2")
                    nc.vector.tensor_scalar_add(m2, mass, 1.0)
                    rg = pAs.tile([128, 1], F32, tag="rg")
                    nc.vector.reciprocal(rg, m2)
                    g_t = pAs.tile([128, 1], F32, tag="gt")
                    nc.vector.tensor_tensor(g_t, mass, rg, ALU.mult)
                    co = pAs.tile([128, 1], F32, tag="co")
                    nc.vector.tensor_scalar(co, g_t, -DECAY, DECAY, ALU.mult, ALU.add)
                    cn = pAs.tile([128, 1], F32, tag="cn")
                    nc.vector.tensor_tensor(cn, g_t, rm, ALU.mult)

                    told = pAs.tile([128, DC], F32, tag="told")
                    nc.vector.tensor_scalar_mul(told, cache_sb[:, base_t + kc, :], co)
                    nc.vector.scalar_tensor_tensor(
                        out=cache_sb[:, base_t + kc, 0:K],
                        in0=ps_ua[kc][:, 2:K + 2], scalar=cn, in1=told[:, 0:K],
                        op0=ALU.mult, op1=ALU.add,
                    )
                    nc.vector.scalar_tensor_tensor(
                        out=cache_sb[:, base_t + kc, K:DC],
                        in0=ps_ub[kc], scalar=cn, in1=told[:, K:DC],
                        op0=ALU.mult, op1=ALU.add,
                    )

            # ---------------- phases C-F ----------------------------------
            with (
                tc.tile_pool(name="woP", bufs=1) as woP,
                tc.tile_pool(name="aoP", bufs=1) as aoP,
            ):
                wo_bf = woP.tile([128, H, DC], BF16)
                nc.gpsimd.dma_start(out=wo_bf, in_=a["Wo"].rearrange("(c p) d -> p c d", p=128))
                wo_f8 = woP.tile([128, H, DC], F8)
                nc.scalar.activation(wo_f8, wo_bf, AF.Copy, scale=WSC)
                aoT = aoP.tile([128, H, N], F8)

                # bo' = (bo + bv @ Wo) * WSC*AOSC (matches ps_o scaling)
                bo_row = woP.tile([1, DC], BF16)
                with tc.tile_pool(name="psBo", bufs=1, space="PSUM") as psBo:
                    ps_bo = psBo.tile([1, DC], F32)
                    for c in range(H):
                        nc.tensor.matmul(ps_bo, bv_cols[:, c:c + 1], wo_bf[:, c, :],
                                         start=(c == 0), stop=(c == H - 1))
                    bo_t = woP.tile([1, DC], F32)
                    nc.vector.tensor_tensor(bo_t, ps_bo, bo_row_f, ALU.add)
                    nc.vector.tensor_scalar_mul(bo_row, bo_t, WSC * AOSC)

                with tc.tile_pool(name="qkvP", bufs=1) as qkvP:
                    qT = qkvP.tile([128, H, N], BF16)
                    kT = qkvP.tile([128, H, N], BF16)
                    v_f8 = qkvP.tile([128, MM, 2, DC], F8)
                    with (
                        tc.tile_pool(name="c2tP", bufs=1) as c2tP,
                        tc.tile_pool(name="wqkvP", bufs=1) as wqkvP,
                    ):
                        # ------- phase C: cache2 -> fp8 d-pair layout -----
                        # c2f8[p, jj, jp, n]: d-chunk j = 2*jj + jp
                        c2f8 = c2tP.tile([128, 2, 2, N], F8)
                        c2bf = c2tP.tile([128, NT, DC], BF16)
                        for t in range(NT):
                            nc.gpsimd.tensor_copy(out=c2bf[:, t, :],
                                                  in_=cache_sb[:, t, :])
                        with tc.tile_pool(name="psC", bufs=2, space="PSUM") as psC:
                            for j in range(4):
                                for tg in range(4):
                                    ps = psC.tile([128, 512], BF16, tag="ctr")
                                    for tt in range(4):
                                        t = tg * 4 + tt
                                        nc.tensor.transpose(
                                            ps[:, tt * 128:(tt + 1) * 128],
                                            c2bf[:, t, j * 128:(j + 1) * 128],
                                            ident_bf,
                                        )
                                    nc.vector.tensor_copy(
                                        out=c2f8[:, j // 2, j % 2, tg * 512:(tg + 1) * 512],
                                        in_=ps)

                        # ------- phase D: q/k/v projections (fp8 DR) ------
                        wq_bf = wqkvP.tile([128, 4, DC], BF16)
                        nc.gpsimd.dma_start(out=wq_bf, in_=a["Wq"].rearrange("(c p) d -> p c d", p=128))
                        wk_bf = wqkvP.tile([128, 4, DC], BF16)
                        nc.gpsimd.dma_start(out=wk_bf, in_=a["Wk"].rearrange("(c p) d -> p c d", p=128))
                        wv_bf = wqkvP.tile([128, 4, DC], BF16)
                        nc.gpsimd.dma_start(out=wv_bf, in_=a["Wv"].rearrange("(c p) d -> p c d", p=128))
                        wq_f8 = wqkvP.tile([128, 4, DC], F8)
                        nc.scalar.activation(wq_f8, wq_bf, AF.Copy, scale=WSC)
                        wk_f8 = wqkvP.tile([128, 4, DC], F8)
                        nc.scalar.activation(wk_f8, wk_bf, AF.Copy, scale=WSC)
                        wv_f8w = wqkvP.tile([128, 4, DC], F8)
                        nc.scalar.activation(wv_f8w, wv_bf, AF.Copy, scale=WSC)

                        with tc.tile_pool(name="psD", bufs=3, space="PSUM") as psD:
                            for dst, w_t, b_t in ((qT, wq_f8, bq_cols), (kT, wk_f8, bk_cols)):
                                for h in range(H):
                                    for c in range(NCH):
                                        ps = psD.tile([128, CL], F32, tag="qk")
                                        for jj in range(2):
                                            nc.tensor.matmul(
                                                ps,
                                                w_t[:, 2 * jj:2 * jj + 2, h * 128:(h + 1) * 128],
                                                c2f8[:, jj, :, c * CL:(c + 1) * CL],
                                                start=(jj == 0), stop=(jj == 1),
                                                perf_mode=DR,
                                            )
                                        nc.vector.tensor_scalar_add(
                                            dst[:, h, c * CL:(c + 1) * CL], ps,
                                            b_t[:, h:h + 1])
                            for m in range(NT):
                                ps = psD.tile([128, DC], F32, tag="v")
                                for jj in range(2):
                                    nc.tensor.matmul(
                                        ps,
                                        c2f8[:, jj, :, m * 128:(m + 1) * 128],
                                        wv_f8w[:, 2 * jj:2 * jj + 2, :],
                                        start=(jj == 0), stop=(jj == 1),
                                        perf_mode=DR,
                                    )
                                nc.scalar.copy(out=v_f8[:, m // 2, m % 2, :], in_=ps)

                    # ------- phase E + pipelined F ------------------------
                    with (
                        tc.tile_pool(name="pEp", bufs=2) as pEp,
                        tc.tile_pool(name="pEs", bufs=2) as pEs,
                        tc.tile_pool(name="pF", bufs=2) as pF,
                        tc.tile_pool(name="pFr", bufs=2) as pFr,
                        tc.tile_pool(name="pFs", bufs=1) as pFs,
                        tc.tile_pool(name="psAtt", bufs=2, space="PSUM") as psAtt,
                        tc.tile_pool(name="psAo", bufs=1, space="PSUM") as psAo,
                        tc.tile_pool(name="psDen", bufs=1, space="PSUM") as psDen,
                        tc.tile_pool(name="psF", bufs=1, space="PSUM") as psF,
                    ):
                        mean_all = pFs.tile([128, NT], F32)
                        ssq_all = pFs.tile([128, NT], F32)

                        def pass1(c, hp, prev_ops):
                            """qk + exp for (c, hp), weaving the previous
                            iteration's completion ops between tiles so the
                            PE and ACT streams both stay dense."""
                            pf8 = pEp.tile([128, MM, 2, 2, CL], F8,
                                           name=f"pf8_{c}_{hp}", tag="pf8")
                            for m in range(NT):
                                mm, mp = m // 2, m % 2
                                ps_a = psAtt.tile([128, 2, CL], F32, tag="att")
                                for h2 in range(2):
                                    h = 2 * hp + h2
                                    nc.tensor.matmul(
                                        ps_a[:, h2, :],
                                        kT[:, h, m * 128:(m + 1) * 128],
                                        qT[:, h, c * CL:(c + 1) * CL],
                                        start=True, stop=True,
                                    )
                                nc.scalar.activation(
                                    pf8[:, mm, mp, :, :], ps_a, AF.Exp,
                                    scale=ATT_SCALE / (WSC * WSC))
                                if prev_ops and m >= 1:
                                    take = (len(prev_ops) + NT - 1 - m) // (NT - m)
                                    for _ in range(take):
                                        prev_ops.pop(0)()
                            while prev_ops:
                                prev_ops.pop(0)()
                            return pf8

                        def make_pass2(c, hp, pf8):
                            """den + att@v over the stored pf8 of (c, hp).
                            Returns (ops, finish): ops are emitted interleaved
                            into the next pass1; finish writes aoT."""
                            ps_den = [psDen.tile([128, CL], F32,
                                                 name=f"dn{h2}_{c}_{hp}",
                                                 tag=f"den{h2}")
                                      for h2 in range(2)]
                            ps_ao = [psAo.tile([128, CL], F32,
                                               name=f"pao{h2}_{c}_{hp}",
                                               tag="ao")
                                     for h2 in range(2)]
                            aoU = [pEs.tile([128, CL], F32,
                                            name=f"aoU{h2}_{c}_{hp}",
                                            tag=f"aoU{h2}")
                                   for h2 in range(2)]
                            den_sb = pEs.tile([1, 2 * CL], F32,
                                              name=f"dsb_{c}_{hp}", tag="densb")
                            rden = pEs.tile([1, 2 * CL], F32,
                                            name=f"rdn_{c}_{hp}", tag="rden")
                            bc_sb = pEs.tile([128, 2 * CL], F32,
                                             name=f"bcs_{c}_{hp}", tag="bcsb")
                            ops = []
                            for h2 in range(2):
                                for mm in range(MM):
                                    ops.append(lambda h2=h2, mm=mm: nc.tensor.matmul(
                                        ps_den[h2][0:32, :], ones2_f8,
                                        pf8[:, mm, :, h2, :],
                                        start=(mm == 0), stop=(mm == MM - 1),
                                        perf_mode=DR))

                            def den_finish():
                                for h2 in range(2):
                                    nc.vector.tensor_copy(
                                        out=den_sb[:, h2 * CL:(h2 + 1) * CL],
                                        in_=ps_den[h2][0:1, :])
                                nc.vector.reciprocal(rden, den_sb)
                                nc.gpsimd.partition_broadcast(bc_sb, rden)
                            ops.append(den_finish)

                            for h2 in range(2):
                                h = 2 * hp + h2
                                for mm in range(MM):
                                    ops.append(lambda h2=h2, h=h, mm=mm: nc.tensor.matmul(
                                        ps_ao[h2],
                                        v_f8[:, mm, :, h * 128:(h + 1) * 128],
                                        pf8[:, mm, :, h2, :],
                                        start=(mm == 0), stop=(mm == MM - 1),
                                        perf_mode=DR))
                                ops.append(lambda h2=h2: nc.vector.tensor_scalar_mul(
                                    aoU[h2], ps_ao[h2], AOSC / WSC))

                            def finish():
                                for h2 in range(2):
                                    h = 2 * hp + h2
                                    nc.vector.scalar_tensor_tensor(
                                        out=aoT[:, h, c * CL:(c + 1) * CL],
                                        in0=aoU[h2], scalar=1.0,
                                        in1=bc_sb[:, h2 * CL:(h2 + 1) * CL],
                                        op0=ALU.mult, op1=ALU.mult)
                            return ops, finish

                        def phase_f(c):
                            r_chunk = pFr.tile([128, NCH, DC], F32, tag="r")
                            for tt in range(NCH):
                                t = c * NCH + tt
                                ps_o = psF.tile([128, DC], F32, tag="o")
                                for jp in range(2):
                                    nc.tensor.matmul(
                                        ps_o,
                                        aoT[:, 2 * jp:2 * jp + 2, t * 128:(t + 1) * 128],
                                        wo_f8[:, 2 * jp:2 * jp + 2, :],
                                        start=(jp == 0), stop=False,
                                        perf_mode=DR,
                                    )
                                nc.tensor.matmul(ps_o, ones_row_bf[:, 0:128], bo_row,
                                                 start=False, stop=True)
                                rsum = pF.tile([128, 1], F32, tag="rsum")
                                nc.vector.scalar_tensor_tensor(
                                    out=r_chunk[:, tt, :], in0=ps_o, scalar=OINV,
                                    in1=cache_sb[:, t, :],
                                    op0=ALU.mult, op1=ALU.add, accum_out=rsum,
                                )
                                nc.vector.tensor_scalar_mul(
                                    mean_all[:, t:t + 1], rsum, 1.0 / DC)
                                sq = pF.tile([128, DC], F32, tag="sq")
                                nc.scalar.activation(
                                    sq, r_chunk[:, tt, :], AF.Square,
                                    accum_out=ssq_all[:, t:t + 1])
                            # var = ssq/DC - mean^2 ; rstd = exp(-0.5*ln(var+eps))
                            t0 = c * NCH
                            mu2 = pF.tile([128, NCH], F32, tag="mu2")
                            nc.vector.tensor_tensor(
                                mu2, mean_all[:, t0:t0 + NCH],
                                mean_all[:, t0:t0 + NCH], ALU.mult)
                            var_t = pF.tile([128, NCH], F32, tag="var")
                            nc.vector.scalar_tensor_tensor(
                                out=var_t, in0=ssq_all[:, t0:t0 + NCH],
                                scalar=1.0 / DC, in1=mu2,
                                op0=ALU.mult, op1=ALU.subtract)
                            lnv = pF.tile([128, NCH], F32, tag="lnv")
                            nc.scalar.activation(lnv, var_t, AF.Ln, bias=eps5_t)
                            rstd = pF.tile([128, NCH], F32, tag="rstd")
                            nc.scalar.activation(rstd, lnv, AF.Exp, scale=-0.5)
                            for tt in range(NCH):
                                t = c * NCH + tt
                                t1 = pF.tile([128, DC], F32, tag="t1")
                                nc.vector.tensor_scalar(
                                    t1, r_chunk[:, tt, :], mean_all[:, t:t + 1],
                                    rstd[:, tt:tt + 1], ALU.subtract, ALU.mult)
                                t2 = pF.tile([128, DC], F32, tag="t2")
                                nc.gpsimd.tensor_tensor(t2, t1, lng_bc, ALU.mult)
                                o_sb = pF.tile([128, DC], F32, tag="osb")
                                nc.gpsimd.tensor_tensor(o_sb, t2, lnb_bc, ALU.add)
                                nc.sync.dma_start(out=out3[:, t, :], in_=o_sb)

                        prev = None  # (c, hp, ops, finish)
                        for c in range(NCH):
                            for hp in range(2):
                                carry = prev[2] if prev is not None else []
                                pf8 = pass1(c, hp, carry)
                                if prev is not None:
                                    prev[3]()
                                    if prev[1] == 1:
                                        phase_f(prev[0])
                                prev = (c, hp, *make_pass2(c, hp, pf8))
                        for op in prev[2]:
                            op()
                        prev[3]()
                        phase_f(NCH - 1)

    nc.compile()
    return nc


_NC_CACHE = {}


def _get_nc():
    if "nc" not in _NC_CACHE:
        _NC_CACHE["nc"] = _build()
    return _NC_CACHE["nc"]


def _in_maps(inputs):
    per_batch = {"y", "cache", "gumbel_u"}
    maps = []
    for b in range(B):
        m = {}
        for name in _INPUT_SPECS:
            arr = np.ascontiguousarray(np.asarray(inputs[name], dtype=np.float32))
            m[name] = arr[b] if name in per_batch else arr
        maps.append(m)
    return maps


def _execute(inputs, trace=False):
    nc = _get_nc()
    res = run_bass_kernel_spmd(nc, _in_maps(inputs), list(range(B)), trace=trace)
    out = np.stack([res.results[b]["out"] for b in range(B)]).astype(np.float32)
    return out, res


def kernel(**inputs) -> np.ndarray:
    out, _ = _execute(inputs)
    return out


# revision 28
# speedup vs baseline: 1.1247x; 1.1247x over previous
"""DLSMN scatter-memory + cache self-attention kernel for Trainium2.

Data-parallel over batch: batch b runs on NeuronCore b (8 cores), no
collectives.  Inside one core (one batch):

  phase A: per 128-token tile of y (bf16): PE-transpose y -> yT chunks,
           fused matmuls [W_write | (W_slot,W_gate)] (bf16), gumbel-softmax
           routing via exp(logits*gamma - ln(-ln(u+eps)+eps)), weighted-
           scatter matmul (f32r) with leading ones columns so write-mass
           comes out of the same accumulation.
  phase B: slot update  upd = (1-g)*DECAY*old + g*updates/(mass+eps).
  phase C: PE-transpose cache2 (f32r data, bf16 identity) -> fp8 chunks
           laid out as DoubleRow d-pairs.
  phase D: q/k/v projections as fp8 DoubleRow matmuls (weights scaled x32
           to avoid fp8 subnormals; scales folded downstream).
  phase E: attention transposed: per (head-pair, 512-col chunk), s^T tiles
           via bf16 matmuls (512-wide moving), exp -> fp8 m-pair tiles;
           att@v and denominators as fp8 DoubleRow over m-pairs, consumed
           one pair late so the PE never stalls on the exp.
  phase F: pipelined one chunk behind E: o-projection fp8 DoubleRow +
           residual + layernorm (variance via ACT Square accumulate).
"""

import numpy as np

import concourse.bacc as bacc
import concourse.mybir as mybir
import concourse.tile as tile
from concourse.bass_utils import run_bass_kernel_spmd
from concourse.masks import make_identity

F32 = mybir.dt.float32
F32R = mybir.dt.float32r
BF16 = mybir.dt.bfloat16
F8 = mybir.dt.float8e4
AF = mybir.ActivationFunctionType
ALU = mybir.AluOpType
DR = mybir.MatmulPerfMode.DoubleRow

B = 8
S = 2048
D = 1024
DC = 512
K = 256
L = 8
H = 4
HD = 128
N = L * K
LAYER_IDX = 3
DECAY = 0.9
EPS = 1e-6
ST = S // 128  # 16 token tiles
NT = N // 128  # 16 slot tiles
DCH = D // 128  # 8 d_model chunks
CL = 512  # attention n-chunk length
NCH = N // CL  # 4 attention chunks
MM = NT // 2  # 8 m-tile pairs
ATT_SCALE = float(1.0 / np.sqrt(np.float32(HD)))
WSC = 32.0  # fp8 weight scale (avoids e4m3 subnormals)
AOSC = 64.0  # aoT fp8 scale
OINV = float(1.0 / (WSC * AOSC))  # o-proj descale (aoT' @ Wo')

_INPUT_SPECS = {
    "y": (S, D), "cache": (N, DC), "gumbel_u": (S, K),
    "W_gate": (D, 1), "b_gate": (1,), "W_slot": (D, K), "b_slot": (K,),
    "gamma": (1,), "W_write": (D, DC), "b_write": (DC,),
    "Wq": (DC, DC), "bq": (DC,), "Wk": (DC, DC), "bk": (DC,),
    "Wv": (DC, DC), "bv": (DC,), "Wo": (DC, DC), "bo": (DC,),
    "ln_g": (DC,), "ln_b": (DC,),
}


def _build():
    nc = bacc.Bacc("TRN2", target_bir_lowering=False, debug=False, num_devices=B)

    a = {
        name: nc.dram_tensor(name, list(shape), F32, kind="ExternalInput").ap()
        for name, shape in _INPUT_SPECS.items()
    }
    out_dram = nc.dram_tensor("out", [N, DC], F32, kind="ExternalOutput").ap()

    y3 = a["y"].rearrange("(t p) d -> p t d", p=128)
    gum3 = a["gumbel_u"].rearrange("(t p) k -> p t k", p=128)
    cache3 = a["cache"].rearrange("(t p) d -> p t d", p=128)
    out3 = out_dram.rearrange("(t p) d -> p t d", p=128)

    with tile.TileContext(nc) as tc:
        with (
            tc.tile_pool(name="const", bufs=1) as const,
            tc.tile_pool(name="cachep", bufs=1) as cachep,
        ):
            ident_bf = const.tile([128, 128], BF16)
            make_identity(nc, ident_bf)
            ident_f = const.tile([128, 128], F32)
            make_identity(nc, ident_f)
            ones_col2_f = const.tile([128, 2], F32)
            nc.vector.memset(ones_col2_f, 1.0)
            ones_row_bf = const.tile([1, DC], BF16)
            nc.vector.memset(ones_row_bf, 1.0)
            ones2_f8 = const.tile([128, 2, 32], F8)
            nc.vector.memset(ones2_f8, 1.0)
            eps8_t = const.tile([128, 1], F32)
            nc.vector.memset(eps8_t, 1e-8)
            eps5_t = const.tile([128, 1], F32)
            nc.vector.memset(eps5_t, 1e-5)
            gamma_t = const.tile([128, 1], F32)
            nc.sync.dma_start(out=gamma_t, in_=a["gamma"].unsqueeze(0).to_broadcast([128, 1]))
            lng_bc = const.tile([128, DC], F32)
            nc.sync.dma_start(out=lng_bc, in_=a["ln_g"].unsqueeze(0).to_broadcast([128, DC]))
            lnb_bc = const.tile([128, DC], F32)
            nc.sync.dma_start(out=lnb_bc, in_=a["ln_b"].unsqueeze(0).to_broadcast([128, DC]))
            bwr_bc = const.tile([128, DC], F32)
            nc.sync.dma_start(out=bwr_bc, in_=a["b_write"].unsqueeze(0).to_broadcast([128, DC]))
            bsg_row = const.tile([1, K + 2], BF16)
            nc.gpsimd.dma_start(out=bsg_row[:, 0:K], in_=a["b_slot"].unsqueeze(0))
            nc.gpsimd.dma_start(out=bsg_row[:, K:K + 1], in_=a["b_gate"].unsqueeze(0))
            nc.gpsimd.dma_start(out=bsg_row[:, K + 1:K + 2], in_=a["b_gate"].unsqueeze(0))
            # q/k per-head bias columns, pre-scaled by WSC (weights are x32)
            bq_raw = const.tile([128, H], F32)
            nc.gpsimd.dma_start(out=bq_raw, in_=a["bq"].rearrange("(h p) -> p h", p=128))
            bq_cols = const.tile([128, H], F32)
            nc.vector.tensor_scalar_mul(bq_cols, bq_raw, WSC)
            bk_raw = const.tile([128, H], F32)
            nc.gpsimd.dma_start(out=bk_raw, in_=a["bk"].rearrange("(h p) -> p h", p=128))
            bk_cols = const.tile([128, H], F32)
            nc.vector.tensor_scalar_mul(bk_cols, bk_raw, WSC)
            bv_cols = const.tile([128, H], BF16)
            nc.gpsimd.dma_start(out=bv_cols, in_=a["bv"].rearrange("(h p) -> p h", p=128))
            bo_row_f = const.tile([1, DC], F32)
            nc.gpsimd.dma_start(out=bo_row_f, in_=a["bo"].unsqueeze(0))

            cache_sb = cachep.tile([128, NT, DC], F32)

            # ---------------- phase A + B: selection & scatter write ------
            with (
                tc.tile_pool(name="wA", bufs=1) as wA,
                tc.tile_pool(name="pA", bufs=2) as pA,
                tc.tile_pool(name="pAs", bufs=3) as pAs,
                tc.tile_pool(name="psU", bufs=1, space="PSUM") as psU,
                tc.tile_pool(name="psA", bufs=1, space="PSUM") as psA,
                tc.tile_pool(name="psT", bufs=2, space="PSUM") as psT,
            ):
                wwr = wA.tile([128, DCH, DC], BF16)
                wsg = wA.tile([128, DCH, K + 2], BF16)

                # gumbel pre-pass: all Ln ops batched
                lnz_all = wA.tile([128, ST, K], F32)
                for i in range(ST):
                    gum = pA.tile([128, K], F32, tag="gum")
                    nc.sync.dma_start(out=gum, in_=gum3[:, i, :])
                    lnu = pAs.tile([128, K], F32, tag="lnu")
                    nc.scalar.activation(lnu, gum, AF.Ln, bias=eps8_t)
                    nc.scalar.activation(lnz_all[:, i, :], lnu, AF.Ln, bias=eps8_t,
                                         scale=-1.0)

                # persistent scatter accumulators: [ones|wv] x w -> [mass|updates]
                ps_ua = [psU.tile([128, K + 2], F32, name=f"ua{kc}", tag=f"ua{kc}")
                         for kc in range(2)]
                ps_ub = [psU.tile([128, K], F32, name=f"ub{kc}", tag=f"ub{kc}")
                         for kc in range(2)]

                pending = []

                def flush_updates():
                    while pending:
                        j, w_j, wv_j = pending.pop(0)
                        for kc in range(2):
                            lhs = w_j[:, kc * 128:(kc + 1) * 128]
                            nc.tensor.matmul(ps_ua[kc], lhs, wv_j[:, 0:K + 2],
                                             start=(j == 0), stop=(j == ST - 1))
                            nc.tensor.matmul(ps_ub[kc], lhs, wv_j[:, K + 2:DC + 2],
                                             start=(j == 0), stop=(j == ST - 1))

                for i in range(ST):
                    y_t = pA.tile([128, D], BF16, tag="y")
                    nc.gpsimd.dma_start(out=y_t, in_=y3[:, i, :])
                    if i == 0:
                        wwr3 = a["W_write"].rearrange("(c p) d -> p c d", p=128)
                        wsl3 = a["W_slot"].rearrange("(c p) k -> p c k", p=128)
                        for c in range(DCH):
                            nc.gpsimd.dma_start(out=wwr[:, c, :], in_=wwr3[:, c, :])
                            nc.gpsimd.dma_start(out=wsg[:, c, 0:K], in_=wsl3[:, c, :])
                        nc.gpsimd.dma_start(out=wsg[:, :, K:K + 1], in_=a["W_gate"].rearrange("(c p) o -> p c o", p=128))
                        nc.gpsimd.dma_start(out=wsg[:, :, K + 1:K + 2], in_=a["W_gate"].rearrange("(c p) o -> p c o", p=128))
                    if i == 1:
                        nc.sync.dma_start(out=cache_sb, in_=cache3)

                    # transpose y tile -> yT (8 chunks of [128d, 128s], bf16)
                    yT = pA.tile([128, D], BF16, tag="yT")
                    for g in range(2):
                        tr = psT.tile([128, 512], BF16, tag="tr")
                        for cc in range(4):
                            c = 4 * g + cc
                            nc.tensor.transpose(
                                tr[:, cc * 128:(cc + 1) * 128],
                                y_t[:, c * 128:(c + 1) * 128],
                                ident_bf,
                            )
                        nc.vector.tensor_copy(out=yT[:, g * 512:(g + 1) * 512], in_=tr)
                    flush_updates()

                    # fused write_vals / (logits, gate) matmuls (bf16)
                    ps_wv = psA.tile([128, DC], F32, tag="wv")
                    for c in range(DCH):
                        nc.tensor.matmul(
                            ps_wv, yT[:, c * 128:(c + 1) * 128], wwr[:, c, :],
                            start=(c == 0), stop=(c == DCH - 1),
                        )
                    ps_lg = psA.tile([128, K + 2], F32, tag="lg")
                    for c in range(DCH):
                        nc.tensor.matmul(
                            ps_lg, yT[:, c * 128:(c + 1) * 128], wsg[:, c, :],
                            start=(c == 0), stop=False,
                        )
                    nc.tensor.matmul(ps_lg, ones_row_bf[:, 0:128], bsg_row,
                                     start=False, stop=True)

                    # t = gamma*logits - lnz
                    t_sb = pAs.tile([128, K], F32, tag="tsb")
                    nc.vector.scalar_tensor_tensor(
                        out=t_sb, in0=ps_lg[:, 0:K], scalar=gamma_t, in1=lnz_all[:, i, :],
                        op0=ALU.mult, op1=ALU.subtract,
                    )

                    # scores = sigmoid(gate) = 1/(1+exp(-gate))
                    sc_e = pAs.tile([128, 1], F32, tag="sce")
                    nc.scalar.activation(sc_e, ps_lg[:, K:K + 1], AF.Exp, scale=-1.0)
                    sc1 = pAs.tile([128, 1], F32, tag="sc1")
                    nc.vector.tensor_scalar_add(sc1, sc_e, 1.0)
                    scores = pAs.tile([128, 1], F32, tag="scores")
                    nc.vector.reciprocal(scores, sc1)

                    # p_unnorm = exp(t), row-sum fused; w = p_unnorm*(scores/rowsum)
                    p_un = pAs.tile([128, K], F32, tag="pun")
                    rs = pAs.tile([128, 1], F32, tag="rs")
                    nc.scalar.activation(p_un, t_sb, AF.Exp, accum_out=rs)
                    rrs = pAs.tile([128, 1], F32, tag="rrs")
                    nc.vector.reciprocal(rrs, rs)
                    w_sb = pAs.tile([128, K], F32R, tag="wsb")
                    nc.vector.tensor_scalar(w_sb, p_un, scores, rrs,
                                            ALU.mult, ALU.mult)

                    # wv_sb = [ones | write_vals + b_write]
                    wv_sb = pAs.tile([128, DC + 2], F32R, tag="wvsb")
                    nc.vector.tensor_copy(out=wv_sb[:, 0:2], in_=ones_col2_f)
                    nc.vector.scalar_tensor_tensor(
                        out=wv_sb[:, 2:DC + 2], in0=ps_wv, scalar=1.0, in1=bwr_bc,
                        op0=ALU.mult, op1=ALU.add,
                    )
                    pending.append((i, w_sb, wv_sb))

                flush_updates()

                # ------- phase B: slot update, overwrite cache rows -------
                base_t = LAYER_IDX * K // 128  # n-tile 6
                for kc in range(2):
                    mass = pAs.tile([128, 1], F32, tag="mass")
                    nc.vector.tensor_copy(out=mass, in_=ps_ua[kc][:, 0:1])
                    m1 = pAs.tile([128, 1], F32, tag="m1")
                    nc.vector.tensor_scalar_add(m1, mass, EPS)
                    rm = pAs.tile([128, 1], F32, tag="rm")
                    nc.vector.reciprocal(rm, m1)
                    m2 = pAs.tile([128, 1], F32, tag="m2")
                    nc.vector.tensor_scalar_add(m2, mass, 1.0)
                    rg = pAs.tile([128, 1], F32, tag="rg")
                    nc.vector.reciprocal(rg, m2)
                    g_t = pAs.tile([128, 1], F32, tag="gt")
                    nc.vector.tensor_tensor(g_t, mass, rg, ALU.mult)
                    co = pAs.tile([128, 1], F32, tag="co")
                    nc.vector.tensor_scalar(co, g_t, -DECAY, DECAY, ALU.mult, ALU.add)
                    cn = pAs.tile([128, 1], F32, tag="cn")
                    nc.vector.tensor_tensor(cn, g_t, rm, ALU.mult)

                    told = pAs.tile([128, DC], F32, tag="told")
                    nc.vector.tensor_scalar_mul(told, cache_sb[:, base_t + kc, :], co)
                    nc.vector.scalar_tensor_tensor(
                        out=cache_sb[:, base_t + kc, 0:K],
                        in0=ps_ua[kc][:, 2:K + 2], scalar=cn, in1=told[:, 0:K],
                        op0=ALU.mult, op1=ALU.add,
                    )
                    nc.vector.scalar_tensor_tensor(
                        out=cache_sb[:, base_t + kc, K:DC],
                        in0=ps_ub[kc], scalar=cn, in1=told[:, K:DC],
                        op0=ALU.mult, op1=ALU.add,
                    )

            # ---------------- phases C-F ----------------------------------
            with (
                tc.tile_pool(name="woP", bufs=1) as woP,
                tc.tile_pool(name="aoP", bufs=1) as aoP,
            ):
                wo_bf = woP.tile([128, H, DC], BF16)
                nc.gpsimd.dma_start(out=wo_bf, in_=a["Wo"].rearrange("(c p) d -> p c d", p=128))
                wo_f8 = woP.tile([128, H, DC], F8)
                nc.scalar.activation(wo_f8, wo_bf, AF.Copy, scale=WSC)
                aoT = aoP.tile([128, H, N], F8)

                # bo' = (bo + bv @ Wo) * WSC*AOSC (matches ps_o scaling)
                bo_row = woP.tile([1, DC], BF16)
                with tc.tile_pool(name="psBo", bufs=1, space="PSUM") as psBo:
                    ps_bo = psBo.tile([1, DC], F32)
                    for c in range(H):
                        nc.tensor.matmul(ps_bo, bv_cols[:, c:c + 1], wo_bf[:, c, :],
                                         start=(c == 0), stop=(c == H - 1))
                    bo_t = woP.tile([1, DC], F32)
                    nc.vector.tensor_tensor(bo_t, ps_bo, bo_row_f, ALU.add)
                    nc.vector.tensor_scalar_mul(bo_row, bo_t, WSC * AOSC)

                with tc.tile_pool(name="qkvP", bufs=1) as qkvP:
                    qT = qkvP.tile([128, H, N], BF16)
                    kT = qkvP.tile([128, H, N], BF16)
                    v_f8 = qkvP.tile([128, MM, 2, DC], F8)
                    with (
                        tc.tile_pool(name="c2tP", bufs=1) as c2tP,
                        tc.tile_pool(name="wqkvP", bufs=1) as wqkvP,
                    ):
                        # ------- phase C: cache2 -> fp8 d-pair layout -----
                        # c2f8[p, jj, jp, n]: d-chunk j = 2*jj + jp
                        c2f8 = c2tP.tile([128, 2, 2, N], F8)
                        with tc.tile_pool(name="psC", bufs=2, space="PSUM") as psC:
                            for j in range(4):
                                for tg in range(4):
                                    ps = psC.tile([128, 512], F32, tag="ctr")
                                    for tt in range(4):
                                        t = tg * 4 + tt
                                        nc.tensor.transpose(
                                            ps[:, tt * 128:(tt + 1) * 128],
                                            cache_sb[:, t, j * 128:(j + 1) * 128],
                                            ident_f,
                                        )
                                    nc.vector.tensor_copy(
                                        out=c2f8[:, j // 2, j % 2, tg * 512:(tg + 1) * 512],
                                        in_=ps)

                        # ------- phase D: q/k/v projections (fp8 DR) ------
                        wq_bf = wqkvP.tile([128, 4, DC], BF16)
                        nc.gpsimd.dma_start(out=wq_bf, in_=a["Wq"].rearrange("(c p) d -> p c d", p=128))
                        wk_bf = wqkvP.tile([128, 4, DC], BF16)
                        nc.gpsimd.dma_start(out=wk_bf, in_=a["Wk"].rearrange("(c p) d -> p c d", p=128))
                        wv_bf = wqkvP.tile([128, 4, DC], BF16)
                        nc.gpsimd.dma_start(out=wv_bf, in_=a["Wv"].rearrange("(c p) d -> p c d", p=128))
                        wq_f8 = wqkvP.tile([128, 4, DC], F8)
                        nc.scalar.activation(wq_f8, wq_bf, AF.Copy, scale=WSC)
                        wk_f8 = wqkvP.tile([128, 4, DC], F8)
                        nc.scalar.activation(wk_f8, wk_bf, AF.Copy, scale=WSC)
                        wv_f8w = wqkvP.tile([128, 4, DC], F8)
                        nc.scalar.activation(wv_f8w, wv_bf, AF.Copy, scale=WSC)

                        with tc.tile_pool(name="psD", bufs=3, space="PSUM") as psD:
                            for dst, w_t, b_t in ((qT, wq_f8, bq_cols), (kT, wk_f8, bk_cols)):
                                for h in range(H):
                                    for c in range(NCH):
                                        ps = psD.tile([128, CL], F32, tag="qk")
                                        for jj in range(2):
                                            nc.tensor.matmul(
                                                ps,
                                                w_t[:, 2 * jj:2 * jj + 2, h * 128:(h + 1) * 128],
                                                c2f8[:, jj, :, c * CL:(c + 1) * CL],
                                                start=(jj == 0), stop=(jj == 1),
                                                perf_mode=DR,
                                            )
                                        nc.vector.tensor_scalar_add(
                                            dst[:, h, c * CL:(c + 1) * CL], ps,
                                            b_t[:, h:h + 1])
                            for m in range(NT):
                                ps = psD.tile([128, DC], F32, tag="v")
                                for jj in range(2):
                                    nc.tensor.matmul(
                                        ps,
                                        c2f8[:, jj, :, m * 128:(m + 1) * 128],
                                        wv_f8w[:, 2 * jj:2 * jj + 2, :],
                                        start=(jj == 0), stop=(jj == 1),
                                        perf_mode=DR,
                                    )
                                nc.scalar.copy(out=v_f8[:, m // 2, m % 2, :], in_=ps)

                    # ------- phase E + pipelined F ------------------------
                    with (
                        tc.tile_pool(name="pEp", bufs=2) as pEp,
                        tc.tile_pool(name="pEs", bufs=2) as pEs,
                        tc.tile_pool(name="pF", bufs=2) as pF,
                        tc.tile_pool(name="pFr", bufs=2) as pFr,
                        tc.tile_pool(name="pFs", bufs=1) as pFs,
                        tc.tile_pool(name="psAtt", bufs=2, space="PSUM") as psAtt,
                        tc.tile_pool(name="psAo", bufs=1, space="PSUM") as psAo,
                        tc.tile_pool(name="psDen", bufs=1, space="PSUM") as psDen,
                        tc.tile_pool(name="psF", bufs=1, space="PSUM") as psF,
                    ):
                        mean_all = pFs.tile([128, NT], F32)
                        ssq_all = pFs.tile([128, NT], F32)

                        def pass1(c, hp, prev_ops):
                            """qk + exp for (c, hp), weaving the previous
                            iteration's completion ops between tiles so the
                            PE and ACT streams both stay dense."""
                            pf8 = pEp.tile([128, MM, 2, 2, CL], F8,
                                           name=f"pf8_{c}_{hp}", tag="pf8")
                            for m in range(NT):
                                mm, mp = m // 2, m % 2
                                ps_a = psAtt.tile([128, 2, CL], F32, tag="att")
                                for h2 in range(2):
                                    h = 2 * hp + h2
                                    nc.tensor.matmul(
                                        ps_a[:, h2, :],
                                        kT[:, h, m * 128:(m + 1) * 128],
                                        qT[:, h, c * CL:(c + 1) * CL],
                                        start=True, stop=True,
                                    )
                                nc.scalar.activation(
                                    pf8[:, mm, mp, :, :], ps_a, AF.Exp,
                                    scale=ATT_SCALE / (WSC * WSC))
                                if prev_ops and m >= 1:
                                    take = (len(prev_ops) + NT - 1 - m) // (NT - m)
                                    for _ in range(take):
                                        prev_ops.pop(0)()
                            while prev_ops:
                                prev_ops.pop(0)()
                            return pf8

                        def make_pass2(c, hp, pf8):
                            """den + att@v over the stored pf8 of (c, hp).
                            Returns (ops, finish): ops are emitted interleaved
                            into the next pass1; finish writes aoT."""
                            ps_den = [psDen.tile([128, CL], F32,
                                                 name=f"dn{h2}_{c}_{hp}",
                                                 tag=f"den{h2}")
                                      for h2 in range(2)]
                            ps_ao = [psAo.tile([128, CL], F32,
                                               name=f"pao{h2}_{c}_{hp}",
                                               tag="ao")
                                     for h2 in range(2)]
                            aoU = [pEs.tile([128, CL], F32,
                                            name=f"aoU{h2}_{c}_{hp}",
                                            tag=f"aoU{h2}")
                                   for h2 in range(2)]
                            den_sb = pEs.tile([1, 2 * CL], F32,
                                              name=f"dsb_{c}_{hp}", tag="densb")
                            rden = pEs.tile([1, 2 * CL], F32,
                                            name=f"rdn_{c}_{hp}", tag="rden")
                            bc_sb = pEs.tile([128, 2 * CL], F32,
                                             name=f"bcs_{c}_{hp}", tag="bcsb")
                            ops = []
                            for h2 in range(2):
                                for mm in range(MM):
                                    ops.append(lambda h2=h2, mm=mm: nc.tensor.matmul(
                                        ps_den[h2][0:32, :], ones2_f8,
                                        pf8[:, mm, :, h2, :],
                                        start=(mm == 0), stop=(mm == MM - 1),
                                        perf_mode=DR))

                            def den_finish():
                                for h2 in range(2):
                                    nc.vector.tensor_copy(
                                        out=den_sb[:, h2 * CL:(h2 + 1) * CL],
                                        in_=ps_den[h2][0:1, :])
                                nc.vector.reciprocal(rden, den_sb)
                                nc.gpsimd.partition_broadcast(bc_sb, rden)
                            ops.append(den_finish)

                            for h2 in range(2):
                                h = 2 * hp + h2
                                for mm in range(MM):
                                    ops.append(lambda h2=h2, h=h, mm=mm: nc.tensor.matmul(
                                        ps_ao[h2],
                                        v_f8[:, mm, :, h * 128:(h + 1) * 128],
                                        pf8[:, mm, :, h2, :],
                                        start=(mm == 0), stop=(mm == MM - 1),
                                        perf_mode=DR))
                                ops.append(lambda h2=h2: nc.vector.tensor_scalar_mul(
                                    aoU[h2], ps_ao[h2], AOSC / WSC))

                            def finish():
                                for h2 in range(2):
                                    h = 2 * hp + h2
                                    nc.vector.scalar_tensor_tensor(
                                        out=aoT[:, h, c * CL:(c + 1) * CL],
                                        in0=aoU[h2], scalar=1.0,
                                        in1=bc_sb[:, h2 * CL:(h2 + 1) * CL],
                                        op0=ALU.mult, op1=ALU.mult)
                            return ops, finish

                        def phase_f(c):
                            ee = nc.vector if c == NCH - 1 else nc.gpsimd
                            r_chunk = pFr.tile([128, NCH, DC], F32, tag="r")
                            for tt in range(NCH):
                                t = c * NCH + tt
                                ps_o = psF.tile([128, DC], F32, tag="o")
                                for jp in range(2):
                                    nc.tensor.matmul(
                                        ps_o,
                                        aoT[:, 2 * jp:2 * jp + 2, t * 128:(t + 1) * 128],
                                        wo_f8[:, 2 * jp:2 * jp + 2, :],
                                        start=(jp == 0), stop=False,
                                        perf_mode=DR,
                                    )
                                nc.tensor.matmul(ps_o, ones_row_bf[:, 0:128], bo_row,
                                                 start=False, stop=True)
                                rsum = pF.tile([128, 1], F32, tag="rsum")
                                nc.vector.scalar_tensor_tensor(
                                    out=r_chunk[:, tt, :], in0=ps_o, scalar=OINV,
                                    in1=cache_sb[:, t, :],
                                    op0=ALU.mult, op1=ALU.add, accum_out=rsum,
                                )
                                nc.vector.tensor_scalar_mul(
                                    mean_all[:, t:t + 1], rsum, 1.0 / DC)
                                sq = pF.tile([128, DC], F32, tag="sq")
                                nc.scalar.activation(
                                    sq, r_chunk[:, tt, :], AF.Square,
                                    accum_out=ssq_all[:, t:t + 1])
                            # var = ssq/DC - mean^2 ; rstd = exp(-0.5*ln(var+eps))
                            t0 = c * NCH
                            mu2 = pF.tile([128, NCH], F32, tag="mu2")
                            nc.vector.tensor_tensor(
                                mu2, mean_all[:, t0:t0 + NCH],
                                mean_all[:, t0:t0 + NCH], ALU.mult)
                            var_t = pF.tile([128, NCH], F32, tag="var")
                            nc.vector.scalar_tensor_tensor(
                                out=var_t, in0=ssq_all[:, t0:t0 + NCH],
                                scalar=1.0 / DC, in1=mu2,
                                op0=ALU.mult, op1=ALU.subtract)
                            lnv = pF.tile([128, NCH], F32, tag="lnv")
                            nc.scalar.activation(lnv, var_t, AF.Ln, bias=eps5_t)
                            rstd = pF.tile([128, NCH], F32, tag="rstd")
                            nc.scalar.activation(rstd, lnv, AF.Exp, scale=-0.5)
                            for tt in range(NCH):
                                t = c * NCH + tt
                                t1 = pF.tile([128, DC], F32, tag="t1")
                                nc.vector.tensor_scalar(
                                    t1, r_chunk[:, tt, :], mean_all[:, t:t + 1],
                                    rstd[:, tt:tt + 1], ALU.subtract, ALU.mult)
                                t2 = pF.tile([128, DC], F32, tag="t2")
                                ee.tensor_tensor(t2, t1, lng_bc, ALU.mult)
                                o_sb = pF.tile([128, DC], F32, tag="osb")
                                ee.tensor_tensor(o_sb, t2, lnb_bc, ALU.add)
                                nc.sync.dma_start(out=out3[:, t, :], in_=o_sb)

                        prev = None  # (c, hp, ops, finish)
                        for c in range(NCH):
                            for hp in range(2):
                                carry = prev[2] if prev is not None else []
                                pf8 = pass1(c, hp, carry)
                                if prev is not None:
                                    prev[3]()
                                    if prev[1] == 1:
                                        phase_f(prev[0])
                                prev = (c, hp, *make_pass2(c, hp, pf8))
                        for op in prev[2]:
                            op()
                        prev[3]()
                        phase_f(NCH - 1)

    nc.compile()
    return nc


_NC_CACHE = {}


def _get_nc():
    if "nc" not in _NC_CACHE:
        _NC_CACHE["nc"] = _build()
    return _NC_CACHE["nc"]


def _in_maps(inputs):
    per_batch = {"y", "cache", "gumbel_u"}
    maps = []
    for b in range(B):
        m = {}
        for name in _INPUT_SPECS:
            arr = np.ascontiguousarray(np.asarray(inputs[name], dtype=np.float32))
            m[name] = arr[b] if name in per_batch else arr
        maps.append(m)
    return maps


def _execute(inputs, trace=False):
    nc = _get_nc()
    res = run_bass_kernel_spmd(nc, _in_maps(inputs), list(range(B)), trace=trace)
    out = np.stack([res.results[b]["out"] for b in range(B)]).astype(np.float32)
    return out, res


def kernel(**inputs) -> np.ndarray:
    out, _ = _execute(inputs)
    return out
